# revision 12
# baseline (speedup 1.0000x reference)
"""Bilateral cross-attention kernel for Trainium2 (8 NeuronCores).

Problem: x,y [2,256,64,64]; four attention branches (i,j) in {1,2}^2:
  f_i = wf_i @ src_i + bf_i   (src_1=x, src_2=y)     [32, 4096]
  g_j = wg_j @ src_j + bg_j                           [32, 4096]
  h_j = wh_j @ x + bh_j       (both h from x)         [32, 4096]
  attn_ij = softmax(f_i^T g_j, axis=-1)               [4096, 4096]
  o_ij = h_j @ attn_ij^T                              [32, 4096]
  contribution to out_i: wo_i[:, half_j] @ (wv_ij @ o_ij + bv_ij)
  x_out = x + contrib_11 + contrib_12 + bo1 ; y_out likewise.

Sharding: 8 cores = 2 batches x 4 branches; each core runs one full
attention with a fused output projection Wc = wo_half @ wv  [256, 32].
Host does only the final residual adds + bias vector adds.

On-core algorithm (flash-style, no max subtraction -- logits are bounded
by ~ +-25 so exp stays in fp32 range):
  S^T chunk [128 keys, 512 q] = g_chunk^T f_qblock  (K=32 row-tiled 3x)
  P = exp(S^T)  on ScalarE (ACT), PSUM->SBUF, one activate per 3 chunks
  acc [33, 512] += [h^T | 1]^T_chunk @ P_chunk   (ones row => softmax sum)
  out = (Wc @ acc[0:32]) * (1/acc[32]) broadcast
All attention matmuls use float32r (1 cycle/row, ~1e-4 relative error).
"""

import os
import numpy as np

BS, C, H, W = 2, 256, 64, 64
N = H * W            # 4096
CH = 32              # qkv channels
QB = 512             # query block (one PSUM bank of fp32)
NQB = N // QB        # 8
KC = 128             # key chunk (partition dim)
NKC = N // KC        # 32
GROUP = 3            # key chunks per round (3 PSUM banks, double buffered)

_CACHE = {}


def _groups():
    gs = []
    c = 0
    while c < NKC:
        gs.append(list(range(c, min(c + GROUP, NKC))))
        c += GROUP
    return gs


def build_nc():
    import concourse.bacc as bacc
    import concourse.mybir as mybir
    import concourse.tile as tile

    F32 = mybir.dt.float32
    F32R = mybir.dt.float32r
    EXP = mybir.ActivationFunctionType.Exp

    nc = bacc.Bacc("TRN2", target_bir_lowering=False)

    xq = nc.dram_tensor("xq", [C, N], F32R, kind="ExternalInput")
    xk = nc.dram_tensor("xk", [C, N], F32R, kind="ExternalInput")
    xv = nc.dram_tensor("xv", [C, N], F32R, kind="ExternalInput")
    wq = nc.dram_tensor("wq", [C, 3 * CH], F32R, kind="ExternalInput")
    wk = nc.dram_tensor("wk", [C, 3 * CH], F32R, kind="ExternalInput")
    wv = nc.dram_tensor("wv", [C, CH + 2], F32R, kind="ExternalInput")
    wc = nc.dram_tensor("wc", [CH, C], F32R, kind="ExternalInput")
    bq = nc.dram_tensor("bq", [3 * CH, 1], F32, kind="ExternalInput")
    bk = nc.dram_tensor("bk", [3 * CH, 1], F32, kind="ExternalInput")
    bh = nc.dram_tensor("bh", [1, CH + 1], F32, kind="ExternalInput")
    out = nc.dram_tensor("out", [C, N], F32, kind="ExternalOutput")

    groups = _groups()

    with tile.TileContext(nc) as tc:
        with (
            tc.tile_pool(name="src", bufs=1) as src,
            tc.tile_pool(name="persist", bufs=1) as persist,
            tc.tile_pool(name="ppool", bufs=3) as ppool,
            tc.tile_pool(name="opool", bufs=4) as opool,
            tc.tile_pool(name="tail", bufs=2) as tailp,
            tc.tile_pool(name="spool", bufs=2, space="PSUM") as spool,
            tc.tile_pool(name="accp", bufs=1, space="PSUM") as accp,
            tc.tile_pool(name="smallp", bufs=1, space="PSUM") as smallp,
        ):
            # ---------------- load inputs ----------------
            xq_sb = src.tile([128, 2, N], F32R)   # [part, ch_chunk, n]
            xk_sb = src.tile([128, 2, N], F32R)
            xv_sb = src.tile([128, 2, N], F32R)
            for cc in range(2):
                nc.sync.dma_start(out=xq_sb[:, cc, :], in_=xq[128 * cc : 128 * cc + 128, :])
                nc.sync.dma_start(out=xk_sb[:, cc, :], in_=xk[128 * cc : 128 * cc + 128, :])
                nc.sync.dma_start(out=xv_sb[:, cc, :], in_=xv[128 * cc : 128 * cc + 128, :])
            wq_sb = persist.tile([128, 2, 3 * CH], F32R)
            wk_sb = persist.tile([128, 2, 3 * CH], F32R)
            wv_sb = persist.tile([128, 2, CH + 2], F32R)
            for cc in range(2):
                nc.sync.dma_start(out=wq_sb[:, cc, :], in_=wq[128 * cc : 128 * cc + 128, :])
                nc.sync.dma_start(out=wk_sb[:, cc, :], in_=wk[128 * cc : 128 * cc + 128, :])
                nc.sync.dma_start(out=wv_sb[:, cc, :], in_=wv[128 * cc : 128 * cc + 128, :])
            wc_sb = persist.tile([CH, C], F32R)
            nc.sync.dma_start(out=wc_sb, in_=wc[:, :])
            bq_sb = persist.tile([3 * CH, 1], F32)
            bk_sb = persist.tile([3 * CH, 1], F32)
            nc.sync.dma_start(out=bq_sb, in_=bq[:, :])
            nc.sync.dma_start(out=bk_sb, in_=bk[:, :])
            # h bias row (+1.0 flag for the ones column) broadcast down partitions
            bh_sb = persist.tile([128, CH + 1], F32)
            nc.gpsimd.dma_start(out=bh_sb, in_=bh[:, :].to_broadcast((128, CH + 1)))

            # ---------------- projections ----------------
            # fq3 / gk3: weights arrive stacked 3x on the host so one
            # matmul writes all three partition blocks (row-tiled S^T
            # matmuls need lhsT/rhs replicated at partition blocks 0..2).
            fq3 = persist.tile([128, N], F32R)
            for n in range(NQB):
                qs = slice(QB * n, QB * n + QB)
                ps_f = spool.tile([128, 1536], F32, tag="s", name="ps_f")
                for cc in range(2):
                    nc.tensor.matmul(
                        out=ps_f[0 : 3 * CH, 0:QB],
                        lhsT=wq_sb[:, cc, :],
                        rhs=xq_sb[:, cc, qs],
                        start=(cc == 0),
                        stop=(cc == 1),
                    )
                nc.vector.tensor_scalar_add(
                    out=fq3[0 : 3 * CH, qs], in0=ps_f[0 : 3 * CH, 0:QB], scalar1=bq_sb
                )

            gk3 = persist.tile([128, N], F32R)
            for n in range(NQB):
                ks = slice(QB * n, QB * n + QB)
                ps_g = spool.tile([128, 1536], F32, tag="s", name="ps_g")
                for cc in range(2):
                    nc.tensor.matmul(
                        out=ps_g[0 : 3 * CH, 0:QB],
                        lhsT=wk_sb[:, cc, :],
                        rhs=xk_sb[:, cc, ks],
                        start=(cc == 0),
                        stop=(cc == 1),
                    )
                nc.vector.tensor_scalar_add(
                    out=gk3[0 : 3 * CH, ks], in0=ps_g[0 : 3 * CH, 0:QB], scalar1=bk_sb
                )

            # haug: h^T chunks + ones column: chunk c at cols [33c, 33c+33)
            # (wv is padded with a zero column; bias row carries the 1.0)
            haug = persist.tile([128, 33 * NKC], F32R)
            for c in range(NKC):
                ks = slice(KC * c, KC * c + KC)
                use_acc = c % 2 == 1
                pool = accp if use_acc else smallp
                ps_h = pool.tile(
                    [128, 512], F32, tag="acc" if use_acc else "sm", name="ps_h"
                )
                for cc in range(2):
                    nc.tensor.matmul(
                        out=ps_h[:, 0 : CH + 2],
                        lhsT=xv_sb[:, cc, ks],
                        rhs=wv_sb[:, cc, :],
                        start=(cc == 0),
                        stop=(cc == 1),
                    )
                # evacuate + add bias (and 1.0 into the ones column) in one op
                nc.vector.tensor_add(
                    out=haug[:, 33 * c : 33 * c + 33],
                    in0=ps_h[:, 0 : CH + 1],
                    in1=bh_sb,
                )

            # ---------------- attention ----------------
            ones32 = persist.tile([1, 32], F32)
            nc.vector.memset(ones32, 1.0)

            # KERNEL_REPEAT > 1 repeats the attention phase (timing only)
            repeat = int(os.environ.get("KERNEL_REPEAT", "1"))
            for n in range(NQB * repeat):
                n = n % NQB
                qs = slice(QB * n, QB * n + QB)
                acc = accp.tile([33, QB], F32, tag="acc", name="acc")
                for s, grp in enumerate(groups):
                    ncols = 512 * len(grp)
                    s_ps = spool.tile([128, 1536], F32, tag="s", name="s_ps")
                    for i, c in enumerate(grp):
                        nc.tensor.matmul(
                            out=s_ps[:, 512 * i : 512 * i + 512],
                            lhsT=gk3[32 * i : 32 * i + 32, KC * c : KC * c + KC],
                            rhs=fq3[32 * i : 32 * i + 32, qs],
                            start=True,
                            stop=True,
                            tile_position=(32 * i, 0),
                        )
                    p_sb = ppool.tile([128, 1536], F32R, tag="p", name="p_sb")
                    nc.scalar.activation(
                        out=p_sb[:, 0:ncols], in_=s_ps[:, 0:ncols], func=EXP
                    )
                    for i, c in enumerate(grp):
                        nc.tensor.matmul(
                            out=acc,
                            lhsT=haug[:, 33 * c : 33 * c + 33],
                            rhs=p_sb[:, 512 * i : 512 * i + 512],
                            start=(c == 0),
                            stop=(c == NKC - 1),
                        )
                # tail: normalize + output projection
                acc_sb = tailp.tile([33, QB], F32, tag="acc_sb", name="acc_sb")
                nc.vector.tensor_copy(out=acc_sb, in_=acc)
                recip = tailp.tile([1, QB], F32, tag="recip", name="recip")
                nc.vector.reciprocal(out=recip, in_=acc_sb[32:33, :])
                rb = smallp.tile([128, 512], F32, tag="sm", name="rb")
                nc.tensor.matmul(
                    out=rb[0:32, :], lhsT=ones32, rhs=recip, start=True, stop=True
                )
                attn_sb = tailp.tile([CH, QB], F32R, tag="attn", name="attn_sb")
                nc.vector.tensor_mul(out=attn_sb, in0=acc_sb[0:32, :], in1=rb[0:32, :])
                for m in range(2):
                    o_ps = smallp.tile([128, 512], F32, tag="sm", name="o_ps")
                    nc.tensor.matmul(
                        out=o_ps,
                        lhsT=wc_sb[:, 128 * m : 128 * m + 128],
                        rhs=attn_sb,
                        start=True,
                        stop=True,
                    )
                    o_sb = opool.tile([128, QB], F32, tag="o", name="o_sb")
                    nc.vector.tensor_copy(out=o_sb, in_=o_ps)
                    nc.sync.dma_start(out=out[128 * m : 128 * m + 128, qs], in_=o_sb)

    nc.compile()
    return nc


def _get_nc():
    if "nc" not in _CACHE:
        _CACHE["nc"] = build_nc()
    return _CACHE["nc"]


def kernel(x, y, wf1, bf1, wg1, bg1, wh1, bh1, wf2, bf2, wg2, bg2, wh2, bh2,
           wv11, bv11, wv12, bv12, wv21, bv21, wv22, bv22, wo1, bo1, wo2, bo2):
    from concourse.bass_utils import run_bass_kernel_spmd

    f32 = np.float32
    x = np.asarray(x, f32)
    y = np.asarray(y, f32)
    wf = {1: np.asarray(wf1, f32), 2: np.asarray(wf2, f32)}
    bf = {1: np.asarray(bf1, f32), 2: np.asarray(bf2, f32)}
    wg = {1: np.asarray(wg1, f32), 2: np.asarray(wg2, f32)}
    bg = {1: np.asarray(bg1, f32), 2: np.asarray(bg2, f32)}
    wh = {1: np.asarray(wh1, f32), 2: np.asarray(wh2, f32)}
    bh_ = {1: np.asarray(bh1, f32), 2: np.asarray(bh2, f32)}
    wvv = {(1, 1): np.asarray(wv11, f32), (1, 2): np.asarray(wv12, f32),
           (2, 1): np.asarray(wv21, f32), (2, 2): np.asarray(wv22, f32)}
    bvv = {(1, 1): np.asarray(bv11, f32), (1, 2): np.asarray(bv12, f32),
           (2, 1): np.asarray(bv21, f32), (2, 2): np.asarray(bv22, f32)}
    wo = {1: np.asarray(wo1, f32), 2: np.asarray(wo2, f32)}
    bo = {1: np.asarray(bo1, f32), 2: np.asarray(bo2, f32)}

    src = {1: x, 2: y}
    branches = [(1, 1), (1, 2), (2, 1), (2, 2)]

    in_maps = []
    for b in range(BS):
        for (i, j) in branches:
            wc_np = wo[i][:, 256 * (j - 1) : 256 * j] @ wvv[(i, j)]  # [256, 32]
            m = {
                "xq": np.ascontiguousarray(src[i][b].reshape(C, N)),
                "xk": np.ascontiguousarray(src[j][b].reshape(C, N)),
                "xv": np.ascontiguousarray(x[b].reshape(C, N)),
                "wq": np.ascontiguousarray(np.tile(wf[i].T, (1, 3))),
                "wk": np.ascontiguousarray(np.tile(wg[j].T, (1, 3))),
                "wv": np.ascontiguousarray(
                    np.concatenate(
                        [wh[j].T, np.zeros((C, 2), f32)], axis=1
                    )
                ),
                "wc": np.ascontiguousarray(wc_np.T),
                "bq": np.ascontiguousarray(np.tile(bf[i].reshape(CH, 1), (3, 1))),
                "bk": np.ascontiguousarray(np.tile(bg[j].reshape(CH, 1), (3, 1))),
                "bh": np.ascontiguousarray(
                    np.concatenate([bh_[j], [1.0]]).astype(f32).reshape(1, CH + 1)
                ),
            }
            in_maps.append(m)

    nc = _get_nc()
    trace = os.environ.get("KERNEL_TRACE", "0")
    kwargs = {}
    if trace == "1":
        kwargs = dict(trace=True, trace_cores=[0])
    elif trace == "all":
        kwargs = dict(trace=True, trace_cores=list(range(8)))
    res = run_bass_kernel_spmd(nc, in_maps, core_ids=list(range(8)), **kwargs)
    _CACHE["last_result"] = res

    parts = {}
    k = 0
    for b in range(BS):
        for (i, j) in branches:
            parts[(b, i, j)] = res.results[k]["out"]
            k += 1

    outs = []
    for i, resid in ((1, x), (2, y)):
        biasvec = (
            wo[i][:, 0:256] @ bvv[(i, 1)]
            + wo[i][:, 256:512] @ bvv[(i, 2)]
            + bo[i]
        ).astype(f32)
        o = np.empty_like(resid)
        for b in range(BS):
            acc = parts[(b, i, 1)] + parts[(b, i, 2)] + biasvec[:, None]
            o[b] = resid[b] + acc.reshape(C, H, W)
        outs.append(o)
    return tuple(outs)
